# revision 33
# baseline (speedup 1.0000x reference)
"""Trainium2 Bass kernel for nn_Conv2d_Local (locally-connected conv, untied
weights).

Problem: x [B=128, 1, 560, 560]; weight [P*NF, 1, 28, 28] with P=39*39=1521
patch locations (stride 14, kernel 28), NF=64 filters; bias [P*NF, 1].
out[b, f*P+p] = sum_{kh,kw} x[b, i*14+kh, j*14+kw] * w[f*P+p, kh, kw] + bias.

Strategy: shard the 39 patch rows across 8 cores (5 rows each, row 39 padded).
Per patch p this is a GEMM patch[b, 784] @ w_p[784, 64]. The contraction is
chunked along kh into per-14-row-period groups of {8,6} rows x 14 kw cols:
K=112 ("A") and K=84 ("B") chunks. 112 = 7x16 spreads each A DMA across all
16 SDMA engines (the DMA splitter assigns ceil(P/16) partitions per engine).
Chunks align to the stride so x slabs are shared between vertically adjacent
patch rows, and adjacent patches' chunks that share an x column-block are
paired into one matmul of N=128 (two 64-wide weight halves -> two adjacent
64-col psum slices), so each x block is loaded stationary exactly once per
patch row.

All device data is bf16 (inputs quantized host-side; psum accumulates fp32;
output stored bf16 and upcast on host). DMA descriptors are kept at ~10KB per
partition (one weight tile per (row, period) covering all 39 patches; one x
slab per period) because SDMA per-engine service rate rises with descriptor
size. Weights ride both HWDGE rings byte-balanced with the x slabs; outputs
ride SWDGE (gpsimd). Outputs drain per psum bank; the last row stores
per-bank so the final DMA is small. Host pre-permutes x and w into these
layouts, adds the bias, and reassembles the final output in fp32.
"""
import sys

if '/opt/trn_rl_repo' not in sys.path:
    sys.path.insert(0, '/opt/trn_rl_repo')

import numpy as np

B = 128
H = W = 560
KH = KW = 28
DH = DW = 14
NF = 64
OH = OW = 39
P = OH * OW
NCORES = 8
NROWS = 5          # patch rows per core (40 total, row 39 is padding)
NPER = 6           # 14-row x periods per core (5 rows + 1 lookahead)
GPER = 41          # global 14-row periods covering 574 (padded) x rows
KA = 112           # A-chunk: 8 kh rows x 14 kw cols
KB = 84            # B-chunk: 6 kh rows x 14 kw cols
WCOLS = OW * 2 * NF  # 4992 weight cols per (row, period, partition)
XA_BUFS = 3
XB_BUFS = 3
WA_BUFS = 12       # per-(row, psum bank) A tiles [112, 2, 1024], 5 per row
WB_BUFS = 15
OROW_BUFS = 2
OT_BUFS = 5
PSUM_BUFS = 8

_CACHE = {}


def build_program(repeats: int = 1):
    import concourse.bacc as bacc
    import concourse.mybir as mybir
    from concourse.tile import TileContext

    f32 = mybir.dt.float32
    bf16 = mybir.dt.bfloat16
    nc = bacc.Bacc("TRN2", target_bir_lowering=False, debug=False,
                   num_devices=NCORES)
    xa_in = nc.dram_tensor("xa", [NPER, KA, 40, 128], bf16, kind="ExternalInput")
    xb_in = nc.dram_tensor("xb", [NPER, KB, 40, 128], bf16, kind="ExternalInput")
    wa_in = nc.dram_tensor("wa", [NROWS, 5, KA, 2, 1024], bf16,
                           kind="ExternalInput")
    wb_in = nc.dram_tensor("wb", [NROWS, 5, KB, 2, 1024], bf16,
                           kind="ExternalInput")
    y_out = nc.dram_tensor("y", [NROWS, 128, OW * NF], bf16, kind="ExternalOutput")

    with TileContext(nc) as tc:
        with tc.tile_pool(name="xa", bufs=XA_BUFS) as xapool, \
             tc.tile_pool(name="xb", bufs=XB_BUFS) as xbpool, \
             tc.tile_pool(name="wa", bufs=WA_BUFS) as wapool, \
             tc.tile_pool(name="wb", bufs=WB_BUFS) as wbpool, \
             tc.tile_pool(name="op", bufs=OROW_BUFS) as opool, \
             tc.tile_pool(name="ot", bufs=OT_BUFS) as otpool, \
             tc.tile_pool(name="ps", bufs=PSUM_BUFS, space="PSUM") as pspool:
            for _rep in range(repeats):
                slabs = {}
                wtiles = {}

                def load_slab(kind, pi):
                    # x slabs ride the SP HWDGE ring; each lands as three
                    # DMAs with 4096/4096/2048-byte per-partition runs (SDMA
                    # serves ~4KB packets at peak rate, 10KB at ~70%)
                    if (kind, pi) in slabs or pi >= NPER:
                        return
                    pool, src, k = ((xapool, xa_in, KA) if kind == 'a'
                                    else (xbpool, xb_in, KB))
                    t = pool.tile([k, 40, 128], bf16, tag=f"x{kind}",
                                  name=f"x{kind}{pi}")
                    for lo, hi in ((0, 16), (16, 32), (32, 40)):
                        nc.sync.dma_start(out=t[:, lo:hi, :],
                                          in_=src[pi, :, lo:hi, :])
                    slabs[(kind, pi)] = t

                def load_w(kind, ri, t5, eng):
                    # one 4KB-per-partition weight tile per (row, psum bank)
                    if ri >= NROWS or (kind, ri, t5) in wtiles:
                        return
                    npat = 8 if t5 < 4 else 7
                    npc = npat * 128
                    pool, src, k = ((wapool, wa_in, KA) if kind == 'a'
                                    else (wbpool, wb_in, KB))
                    wt = pool.tile([k, 2, 1024], bf16, tag=f"w{kind}",
                                   name=f"w{kind}{ri}_{t5}")
                    eng.dma_start(out=wt[:, :, :npc],
                                  in_=src[ri, t5, :, :, :npc])
                    wtiles[(kind, ri, t5)] = wt

                # prologue in demand order. Ring byte balance: SP carries the
                # x slabs plus B tiles t5 in {0,1,4} for rows 0-3 (~16.0MB),
                # ACT carries all A tiles, B t5 in {2,3}, and ALL of the last
                # row's B tiles (~15.6MB) so the final tiles arrive in exact
                # demand order on one ring. B tiles get a full row of
                # prefetch lead.
                load_slab('a', 0)
                load_slab('b', 0)
                for t5 in (2, 3, 4):
                    load_w('b', 0, t5, nc.sync)
                load_slab('a', 1)
                load_slab('b', 1)
                for t5 in (2, 3, 4):
                    load_w('b', 1, t5, nc.sync)
                load_slab('a', 2)
                load_slab('b', 2)

                for ri in range(NROWS):
                    if ri >= 1:
                        if ri + 1 <= NROWS - 2:
                            for t5 in (2, 3, 4):
                                load_w('b', ri + 1, t5, nc.sync)
                        load_slab('a', ri + 2)
                        load_slab('b', ri + 2)
                    last_row = (ri == NROWS - 1)
                    if not last_row:
                        orow = opool.tile([128, OW * NF], bf16, tag="orow",
                                          name=f"orow{ri}")
                    psrow = [pspool.tile([128, 512], f32, tag="ps",
                                         name=f"ps{ri}_{t5}")
                             for t5 in range(5)]
                    # sub-pass order: (A, per0), (A, per1), (B, per0),
                    # (B, per1) — B tiles arrive later on the rings, so the
                    # B passes run last and late tiles stall less FIFO work
                    for g4 in range(4):
                        kind = 'ab'[g4 // 2]
                        per = g4 % 2
                        slab = slabs[(kind, ri + per)]
                        for t5 in range(5):
                            p0 = 8 * t5
                            npat = 8 if t5 < 4 else 7
                            if g4 == 0:
                                load_w('a', ri, t5, nc.scalar)
                                if t5 in (0, 1):
                                    if ri == 0:
                                        load_w('b', 0, t5, nc.scalar)
                                    load_w('b', ri + 1, t5, nc.scalar)
                                if last_row and t5 == 4:
                                    for t in range(5):
                                        load_w('b', ri, t, nc.scalar)
                            wt = wtiles[(kind, ri, t5)]
                            for mrel in range(npat + 1):
                                m = p0 + mrel
                                if mrel == 0:
                                    wsl = (0, 64)
                                    osl = (0, 64)
                                elif mrel < npat:
                                    wsl = (128 * mrel - 64, 128 * mrel + 64)
                                    osl = (64 * (mrel - 1), 64 * (mrel + 1))
                                else:
                                    wsl = (128 * npat - 64, 128 * npat)
                                    osl = (64 * (npat - 1), 64 * npat)
                                start = (g4 == 0 and mrel == 0)
                                stop = (g4 == 3 and mrel == npat)
                                nc.tensor.matmul(
                                    psrow[t5][:, osl[0]:osl[1]],
                                    slab[:, m, :],
                                    wt[:, per, wsl[0]:wsl[1]],
                                    start=start, stop=stop)
                            if g4 == 3:
                                # drain this bank now: the cast overlaps the
                                # remaining banks' matmuls; the last row also
                                # stores per-bank so the final DMA is small
                                if last_row:
                                    ot = otpool.tile([128, 512], bf16,
                                                     tag="ot",
                                                     name=f"o{ri}_{t5}")
                                    nc.vector.tensor_copy(
                                        out=ot[:, :npat * 64],
                                        in_=psrow[t5][:, :npat * 64])
                                    nc.gpsimd.dma_start(
                                        out=y_out[ri, :,
                                                  512 * t5: 512 * t5 + npat * 64],
                                        in_=ot[:, :npat * 64])
                                else:
                                    nc.vector.tensor_copy(
                                        out=orow[:, 512 * t5: 512 * t5 + npat * 64],
                                        in_=psrow[t5][:, :npat * 64])
                    if not last_row:
                        nc.gpsimd.dma_start(out=y_out[ri], in_=orow)
    nc.finalize()
    return nc


def _preprocess(x, weight):
    """Build per-core bf16 input maps from full x [B,1,560,560],
    weight [P*NF,1,28,28]."""
    import ml_dtypes
    bf16 = ml_dtypes.bfloat16

    x = np.asarray(x, dtype=np.float32).astype(bf16)
    weight = np.asarray(weight, dtype=np.float32).astype(bf16)

    # x -> pixel-major [574(pad), 560, 128], then per-14-row-period slabs of
    # {8,6} rows with partition order (row_in_group, kw'):
    #   xA [41, 112, 40, 128], xB [41, 84, 40, 128]
    xt = np.zeros((GPER * 14, W, B), dtype=bf16)
    xt[:H] = x[:, 0].transpose(1, 2, 0)
    x6 = xt.reshape(GPER, 14, 40, 14, B)
    xa = np.ascontiguousarray(
        x6[:, :8].transpose(0, 1, 3, 2, 4)).reshape(GPER, KA, 40, 128)
    xb = np.ascontiguousarray(
        x6[:, 8:].transpose(0, 1, 3, 2, 4)).reshape(GPER, KB, 40, 128)

    # weight rows are f*P + p; reshape kh=(period, r), kw=(delta, kw') and
    # order as [i, k=(r,kw'), period, cols=(j, delta, f)] for r in the A (8)
    # and B (6) row groups, then split cols into 8-patch psum-bank blocks
    w7 = weight.reshape(NF, OH, OW, 2, 14, 2, 14)  # [f,i,j,per,r,delta,kw']
    wa_flat = np.ascontiguousarray(
        w7[:, :, :, :, :8].transpose(1, 4, 6, 3, 2, 5, 0)
    ).reshape(OH, KA, 2, WCOLS)
    wb_flat = np.ascontiguousarray(
        w7[:, :, :, :, 8:].transpose(1, 4, 6, 3, 2, 5, 0)
    ).reshape(OH, KB, 2, WCOLS)

    wa_dev = np.zeros((NROWS * NCORES, 5, KA, 2, 1024), dtype=bf16)
    wb_dev = np.zeros((NROWS * NCORES, 5, KB, 2, 1024), dtype=bf16)
    for t5 in range(5):
        p0 = 8 * t5
        npat = 8 if t5 < 4 else 7
        wa_dev[:OH, t5, :, :, :npat * 128] = \
            wa_flat[:, :, :, 128 * p0: 128 * (p0 + npat)]
        wb_dev[:OH, t5, :, :, :npat * 128] = \
            wb_flat[:, :, :, 128 * p0: 128 * (p0 + npat)]

    in_maps = []
    for c in range(NCORES):
        # core c covers periods 5c..5c+5; core 7's last period is 40 = GPER-1
        in_maps.append({
            "xa": np.ascontiguousarray(xa[5 * c: 5 * c + NPER]),
            "xb": np.ascontiguousarray(xb[5 * c: 5 * c + NPER]),
            "wa": np.ascontiguousarray(wa_dev[NROWS * c: NROWS * (c + 1)]),
            "wb": np.ascontiguousarray(wb_dev[NROWS * c: NROWS * (c + 1)]),
        })
    return in_maps


def _postprocess(results, bias):
    """results: list of per-core dicts with 'y' [NROWS, 128, OW*NF] bf16."""
    y = np.stack([np.asarray(r["y"], dtype=np.float32) for r in results])
    y = y.reshape(NCORES * NROWS, B, OW, NF)[:OH]    # [39, 128, 39, 64]
    out = np.ascontiguousarray(y.transpose(1, 3, 0, 2)).reshape(B, NF * P)
    out = out + np.asarray(bias, dtype=np.float32).reshape(1, NF * P)
    return out.reshape(B, NF * P, 1)


def kernel(x, weight, bias):
    from concourse.bass_utils import run_bass_kernel_spmd

    if "nc" not in _CACHE:
        _CACHE["nc"] = build_program()
    nc = _CACHE["nc"]
    in_maps = _preprocess(x, weight)
    res = run_bass_kernel_spmd(nc, in_maps, core_ids=list(range(NCORES)))
    return _postprocess(res.results, bias)
